# revision 57
# baseline (speedup 1.0000x reference)
"""Trainium2 Bass kernel for the fuzzy joint-membership layer.

Math (derived from the reference 2-qubit circuit, verified vs oracle):
  out[b, 2p,   c] = 0.5 + 0.5*cos(theta_c)*cos(x0) - 0.5*sin(theta_c)*sin(x0)*sin(x1)
  out[b, 2p+1, c] = 0.5 + 0.5*cos(x0)*cos(x1)
where x0 = xf[b, pair_idx[b,p,0]], x1 = xf[b, pair_idx[b,p,1]].

Sharding: pure data parallel, batch 4096 -> 8 cores x 512 rows.

Gather strategy: the per-row gather xf[b, idx[b,j]] is expressed as
gpsimd local_scatter (hardware vector scatter in Q7 local RAM, with
per-partition independent indices) instead of ap_gather (which costs
~36 cycles per index on the Q7 command interface):
  - host precomputes, per row, for each pixel the FIRST slot j wanting
    it (idxA[row, pix] = j or -1) plus log-doubling duplicate maps
    CT[t][row, src_slot] = dst_slot covering ordinals [2^t, 2^{t+1})
  - S0 = scatter(x16, idxA); U = S0
  - round t: St = scatter(U, CT[t]); U = U + St (disjoint, exact fp16)
x is moved in fp16 (abs err <= ~1.5e-3 after trig, vs 2e-2 tolerance).

Slot layout is half-split (x0 of pair p -> slot p, x1 -> slot 460+p)
so all downstream pair reads are unit-stride (DVE 2x fp16 mode).
Intermediates and the output tile are fp16 (host upcasts to f32;
error budget ~4e-3 vs the 2e-2 gate). Range reduction (magic round)
runs on ACT, Sin on ACT, products + class expansion on DVE with
broadcast APs, odd block replicated by one broadcast ACT copy.
"""

import math
import numpy as np

B, PIX, NPAIR, C = 4096, 3072, 460, 10
NG = 2 * NPAIR          # 920 gathered values per row
OUTW = NG * C           # 9200
NCORES = 8
BS = B // NCORES        # 512 rows per core
TILES = BS // 128       # 4

_cache = {}


def _ensure_path():
    try:
        import concourse  # noqa: F401
    except ImportError:
        import sys
        sys.path.insert(0, "/opt/trn_rl_repo")


def build_nc(bs=BS, rounds=3):
    _ensure_path()
    from contextlib import ExitStack
    import concourse.tile as tile
    from concourse import bacc, mybir

    f32, f16, i16 = mybir.dt.float32, mybir.dt.float16, mybir.dt.int16
    bf16 = mybir.dt.bfloat16
    Sin = mybir.ActivationFunctionType.Sin
    Copy = mybir.ActivationFunctionType.Copy
    Abs = mybir.ActivationFunctionType.Abs
    mult = mybir.AluOpType.mult
    add = mybir.AluOpType.add
    sub_ = mybir.AluOpType.subtract
    maxop = mybir.AluOpType.max
    ntiles = bs // 128

    nc = bacc.Bacc("TRN2", target_bir_lowering=False, debug=False)
    x_ext = nc.declare_dram_parameter("x16", [bs, PIX], f16, isOutput=False)
    ia_ext = nc.declare_dram_parameter("ia", [bs, PIX], i16, isOutput=False)
    cc_ext = nc.declare_dram_parameter("cc", [bs, rounds * NG], i16, isOutput=False)
    th_ext = nc.declare_dram_parameter("theta", [128, C], f32, isOutput=False)
    out_ext = nc.declare_dram_parameter("out", [bs, OUTW], f16, isOutput=True)

    PI, TWO_PI = math.pi, 2 * math.pi
    MAGIC, INV2PI = 1.5 * 2 ** 23, 1.0 / (2 * math.pi)

    with tile.TileContext(nc) as tc, ExitStack() as ctx:
        cpool = ctx.enter_context(tc.tile_pool(name="const", bufs=1))
        xpool = ctx.enter_context(tc.tile_pool(name="xf", bufs=2))
        ipool = ctx.enter_context(tc.tile_pool(name="ia", bufs=2))
        kpool = ctx.enter_context(tc.tile_pool(name="cc", bufs=2))
        spool = ctx.enter_context(tc.tile_pool(name="sc", bufs=2))
        upool = ctx.enter_context(tc.tile_pool(name="uc", bufs=2))
        vpool = ctx.enter_context(tc.tile_pool(name="v", bufs=2))
        tpool = ctx.enter_context(tc.tile_pool(name="trig", bufs=2))
        wpool = ctx.enter_context(tc.tile_pool(name="we", bufs=2))
        epool = ctx.enter_context(tc.tile_pool(name="expand", bufs=2))
        opool = ctx.enter_context(tc.tile_pool(name="ot", bufs=2))

        pihalf = cpool.tile([128, 1], f32)
        nc.vector.memset(pihalf[:], PI / 2)
        zerob = cpool.tile([128, 1], f32)
        nc.vector.memset(zerob[:], 0.0)

        # Scalar-engine Sin only accepts [-pi, pi]. Range-reduce with the
        # round-to-nearest magic trick: n = (v/2pi + M) - M, -r = 2pi*n - v.
        # Then -sin(v) = Sin(-r) and cos(v) = Sin(pi/2 - |r|); the sin sign
        # flip cancels in sin*sin products and is absorbed into nhst.
        def trig(pool, src, width, tagp, on_act):
            """returns (cv, svN) = (cos(src), -sin(src)), width cols."""
            t1 = pool.tile([128, width], f32, tag=tagp + "t1")
            if on_act:
                nc.scalar.activation(t1[:], src, Copy, bias=MAGIC, scale=INV2PI)
                nc.scalar.activation(t1[:], t1[:], Copy, bias=-MAGIC, scale=1.0)
            else:
                nc.vector.tensor_scalar(t1[:], src, INV2PI, MAGIC, mult, add)
                nc.vector.tensor_scalar(t1[:], t1[:], MAGIC, None, sub_)
            negr = pool.tile([128, width], f16, tag=tagp + "negr")
            nc.vector.scalar_tensor_tensor(negr[:], t1[:], TWO_PI, src, mult, sub_)
            absr = pool.tile([128, width], f16, tag=tagp + "absr")
            nc.vector.tensor_scalar(absr[:], negr[:], -1.0, None, mult)
            nc.vector.tensor_tensor(absr[:], absr[:], negr[:], maxop)  # |r|
            cv = pool.tile([128, width], f16, tag=tagp + "cv")
            svN = pool.tile([128, width], f16, tag=tagp + "svN")
            nc.scalar.activation(svN[:], negr[:], Sin, bias=zerob[:, 0:1])
            nc.scalar.activation(cv[:], absr[:], Sin, bias=pihalf[:, 0:1], scale=-1.0)
            return cv, svN

        # theta coefficients: hct = 0.5*cos(theta), nhst = -0.5*sin(theta)
        th_sb = cpool.tile([128, C], f32)
        nc.sync.dma_start(out=th_sb[:], in_=th_ext[:, :])
        tt1 = cpool.tile([128, C], f32)
        nc.vector.tensor_scalar(tt1[:], th_sb[:], INV2PI, MAGIC, mult, add)
        nc.vector.tensor_scalar(tt1[:], tt1[:], MAGIC, None, sub_)
        tnegr = cpool.tile([128, C], f32)
        nc.vector.scalar_tensor_tensor(tnegr[:], tt1[:], TWO_PI, th_sb[:], mult, sub_)
        nc.vector.tensor_scalar(tt1[:], tnegr[:], -1.0, None, mult)
        nc.vector.tensor_tensor(tt1[:], tt1[:], tnegr[:], maxop)
        cvt = cpool.tile([128, C], f32)
        svNt = cpool.tile([128, C], f32)
        nc.scalar.activation(svNt[:], tnegr[:], Sin, bias=zerob[:, 0:1])
        nc.scalar.activation(cvt[:], tt1[:], Sin, bias=pihalf[:, 0:1], scale=-1.0)
        hcoef = cpool.tile([128, 2 * C], f32)
        nc.vector.tensor_scalar(hcoef[:, 0:C], cvt[:], 0.5, None, mult)
        nc.vector.tensor_scalar(hcoef[:, C:2 * C], svNt[:], 0.5, None, mult)
        hct = hcoef[:, 0:C]        # 0.5*cos(theta)
        nhst = hcoef[:, C:2 * C]   # -0.5*sin(theta) = 0.5*svN

        # pair-major replicated theta tables (one-time, via ACT): unit
        # stride operands let the per-tile products hit the 16-bit 2x mode
        hrep = cpool.tile([128, NPAIR * C], bf16)
        nrep = cpool.tile([128, NPAIR * C], bf16)
        nc.scalar.activation(
            hrep[:].rearrange("p (a b) -> p a b", b=C),
            hct.unsqueeze(1).broadcast_to([128, NPAIR, C]), Copy,
        )
        nc.scalar.activation(
            nrep[:].rearrange("p (a b) -> p a b", b=C),
            nhst.unsqueeze(1).broadcast_to([128, NPAIR, C]), Copy,
        )

        for t in range(ntiles):
            rows = slice(t * 128, (t + 1) * 128)
            ia = ipool.tile([128, PIX], i16)
            xf = xpool.tile([128, PIX], f16)
            if t == 0:
                # split the first tile's DMAs and round-0 scatter into
                # pixel halves so work starts before the full tile lands
                HX = PIX // 2
                nc.sync.dma_start(out=ia[:, 0:HX], in_=ia_ext[rows, 0:HX])
                nc.sync.dma_start(out=xf[:, 0:HX], in_=x_ext[rows, 0:HX])
                nc.sync.dma_start(out=ia[:, HX:PIX], in_=ia_ext[rows, HX:PIX])
                nc.sync.dma_start(out=xf[:, HX:PIX], in_=x_ext[rows, HX:PIX])
            else:
                nc.sync.dma_start(out=ia[:], in_=ia_ext[rows, :])
                nc.sync.dma_start(out=xf[:], in_=x_ext[rows, :])
            ct = kpool.tile([128, rounds * NG], i16)
            nc.sync.dma_start(out=ct[:], in_=cc_ext[rows, :])

            # log-doubling scatter rounds; U accumulates (disjoint supports)
            S = spool.tile([128, (rounds + 1) * NG], f16)
            U = upool.tile([128, rounds * NG], f16)
            if t == 0:
                HX = PIX // 2
                s0b = vpool.tile([128, NG], f16, tag="s0b")
                nc.gpsimd.local_scatter(
                    S[:, 0:NG], xf[:, 0:HX], ia[:, 0:HX],
                    channels=128, num_elems=NG, num_idxs=HX,
                )
                nc.gpsimd.local_scatter(
                    s0b[:], xf[:, HX:PIX], ia[:, HX:PIX],
                    channels=128, num_elems=NG, num_idxs=HX,
                )
                nc.vector.tensor_tensor(S[:, 0:NG], S[:, 0:NG], s0b[:], add)
            else:
                nc.gpsimd.local_scatter(
                    S[:, 0:NG], xf[:], ia[:],
                    channels=128, num_elems=NG, num_idxs=PIX,
                )
            V = vpool.tile([128, NG], f16)
            for r in range(1, rounds + 1):
                src = S[:, 0:NG] if r == 1 else U[:, (r - 2) * NG:(r - 1) * NG]
                nc.gpsimd.local_scatter(
                    S[:, r * NG:(r + 1) * NG], src,
                    ct[:, (r - 1) * NG:(r - 1) * NG + NG],
                    channels=128, num_elems=NG, num_idxs=NG,
                )
                dst = V[:] if r == rounds else U[:, (r - 1) * NG:r * NG]
                nc.vector.tensor_tensor(dst, src, S[:, r * NG:(r + 1) * NG], add)

            cv, sv = trig(tpool, V[:], NG, "g", True)

            # half-split layout: slots [0:460] = x0, [460:920] = x1
            w = wpool.tile([128, NPAIR], f16, tag="w")
            e = wpool.tile([128, NPAIR], f16, tag="e")
            nc.vector.tensor_tensor(w[:], sv[:, 0:NPAIR], sv[:, NPAIR:NG], mult)
            nc.vector.tensor_tensor(e[:], cv[:, 0:NPAIR], cv[:, NPAIR:NG], mult)

            # class expansion: even = (A*hct_c + 0.5) + W*nhst_c, odd = 0.5*E+0.5
            # ACT (non-contending) replicates A and W pair-major; the DVE
            # products are then all-unit-stride 16-bit (2x mode), in-place.
            # Processed in pair-halves so ACT/DVE/DMA pipeline in the drain.
            tev = epool.tile([128, NPAIR * C], bf16, tag="tev")
            tw2 = epool.tile([128, NPAIR * C], bf16, tag="tw2")
            ot = opool.tile([128, OUTW], f16)
            otv = ot[:].rearrange("p (a b) -> p a b", b=2 * C)
            HP = NPAIR // 4
            for h in range(4):
                ph = slice(h * HP, (h + 1) * HP)
                pc = slice(h * HP * C, (h + 1) * HP * C)
                tev3 = tev[:, pc].rearrange("p (a b) -> p a b", b=C)
                tw23 = tw2[:, pc].rearrange("p (a b) -> p a b", b=C)
                A3 = cv[:, ph].unsqueeze(2).broadcast_to([128, HP, C])
                W3 = w[:, ph].unsqueeze(2).broadcast_to([128, HP, C])
                E3 = e[:, ph].unsqueeze(2).broadcast_to([128, HP, C])
                # odd block first: it only needs e, so the chunk DMA ends
                # up gated by the stt alone
                nc.scalar.activation(otv[:, ph, C:2 * C], E3, Copy, bias=0.5, scale=0.5)
                nc.scalar.activation(tev3, A3, Copy)
                nc.vector.tensor_tensor(tev[:, pc], tev[:, pc], hrep[:, pc], mult)
                nc.scalar.activation(tw23, W3, Copy)
                nc.vector.tensor_tensor(tw2[:, pc], tw2[:, pc], nrep[:, pc], mult)
                nc.vector.scalar_tensor_tensor(
                    otv[:, ph, 0:C], tev3, 0.5, tw23, add, add
                )
                nc.sync.dma_start(
                    out=out_ext[rows, h * HP * 2 * C:(h + 1) * HP * 2 * C],
                    in_=ot[:, h * HP * 2 * C:(h + 1) * HP * 2 * C],
                )

    nc.compile()
    return nc


def _prep_scatter_maps(pair_idx):
    """Build round-0 scatter map and log-doubling duplicate maps.

    Slot layout is half-split: x0 of pair p -> slot p, x1 -> slot 460+p.
    Chain round 0 serves ordinal 1, round 1 serves ordinals 2-3 (via the
    running union), round 2 serves 4-7, etc.
    Returns (idxA [B, PIX] i16, chains [T, B, NG] i16, T).
    """
    pidx = pair_idx.reshape(B, NPAIR, 2)
    idx = np.concatenate([pidx[:, :, 0], pidx[:, :, 1]], axis=1).astype(np.int64)
    j = np.arange(NG, dtype=np.int64)[None, :]
    ordk = np.argsort(idx * 1024 + j, axis=1)      # slots sorted by (pixel, slot)
    px_sorted = np.take_along_axis(idx, ordk, axis=1)
    first = np.ones((B, NG), dtype=bool)
    first[:, 1:] = px_sorted[:, 1:] != px_sorted[:, :-1]
    kk = np.broadcast_to(np.arange(NG, dtype=np.int64), (B, NG))
    run_start = np.maximum.accumulate(np.where(first, kk, 0), axis=1)
    o = kk - run_start                              # occurrence ordinal per sorted pos
    maxmult = int(o.max()) + 1
    T = max(1, (maxmult - 1).bit_length())          # ceil(log2(maxmult))

    idxA = np.full((B, PIX), -1, np.int16)
    rr, cc = np.nonzero(first)
    idxA[rr, px_sorted[rr, cc]] = ordk[rr, cc]

    chains = np.full((T, B, NG), -1, np.int16)
    rr, cc = np.nonzero(o >= 1)
    oo = o[rr, cc]
    t_of = np.zeros_like(oo)
    for t in range(1, T):
        t_of[oo >= (1 << t)] = t
    src = ordk[rr, cc - (1 << t_of)]
    dst = ordk[rr, cc]
    chains[t_of, rr, src] = dst
    return idxA, chains, T


def _get_nc(rounds):
    key = ("nc", rounds)
    if key not in _cache:
        _cache[key] = build_nc(rounds=rounds)
    return _cache[key]


def kernel(x, pair_idx, theta):
    _ensure_path()
    from concourse.bass_utils import run_bass_kernel_spmd

    x16 = np.ascontiguousarray(
        np.asarray(x, dtype=np.float32).reshape(B, PIX).astype(np.float16)
    )
    idxA, chains, T = _prep_scatter_maps(np.asarray(pair_idx))
    nc = _get_nc(T)
    cc = np.ascontiguousarray(
        chains.transpose(1, 0, 2).reshape(B, T * NG)
    )
    thb = np.ascontiguousarray(
        np.tile(np.asarray(theta, dtype=np.float32).reshape(1, C), (128, 1))
    )
    in_maps = [
        {
            "x16": x16[k * BS:(k + 1) * BS],
            "ia": idxA[k * BS:(k + 1) * BS],
            "cc": cc[k * BS:(k + 1) * BS],
            "theta": thb,
        }
        for k in range(NCORES)
    ]
    res = run_bass_kernel_spmd(nc, in_maps, list(range(NCORES))).results
    out = np.concatenate(
        [res[k]["out"].astype(np.float32) for k in range(NCORES)], axis=0
    )
    return out.reshape(B, NG, C)


# revision 58
# speedup vs baseline: 1.0369x; 1.0369x over previous
"""Trainium2 Bass kernel for the fuzzy joint-membership layer.

Math (derived from the reference 2-qubit circuit, verified vs oracle):
  out[b, 2p,   c] = 0.5 + 0.5*cos(theta_c)*cos(x0) - 0.5*sin(theta_c)*sin(x0)*sin(x1)
  out[b, 2p+1, c] = 0.5 + 0.5*cos(x0)*cos(x1)
where x0 = xf[b, pair_idx[b,p,0]], x1 = xf[b, pair_idx[b,p,1]].

Sharding: pure data parallel, batch 4096 -> 8 cores x 512 rows.

Gather strategy: the per-row gather xf[b, idx[b,j]] is expressed as
gpsimd local_scatter (hardware vector scatter in Q7 local RAM, with
per-partition independent indices) instead of ap_gather (which costs
~36 cycles per index on the Q7 command interface):
  - host precomputes, per row, for each pixel the FIRST slot j wanting
    it (idxA[row, pix] = j or -1) plus log-doubling duplicate maps
    CT[t][row, src_slot] = dst_slot covering ordinals [2^t, 2^{t+1})
  - S0 = scatter(x16, idxA); U = S0
  - round t: St = scatter(U, CT[t]); U = U + St (disjoint, exact fp16)
x is moved in fp16 (abs err <= ~1.5e-3 after trig, vs 2e-2 tolerance).

Slot layout is half-split (x0 of pair p -> slot p, x1 -> slot 460+p)
so all downstream pair reads are unit-stride (DVE 2x fp16 mode).
Intermediates and the output tile are fp16 (host upcasts to f32;
error budget ~4e-3 vs the 2e-2 gate). Range reduction (magic round)
runs on ACT, Sin on ACT, products + class expansion on DVE with
broadcast APs, odd block replicated by one broadcast ACT copy.
"""

import math
import numpy as np

B, PIX, NPAIR, C = 4096, 3072, 460, 10
NG = 2 * NPAIR          # 920 gathered values per row
OUTW = NG * C           # 9200
NCORES = 8
BS = B // NCORES        # 512 rows per core
TILES = BS // 128       # 4

_cache = {}


def _ensure_path():
    try:
        import concourse  # noqa: F401
    except ImportError:
        import sys
        sys.path.insert(0, "/opt/trn_rl_repo")


def build_nc(bs=BS, rounds=3):
    _ensure_path()
    from contextlib import ExitStack
    import concourse.tile as tile
    from concourse import bacc, mybir

    f32, f16, i16 = mybir.dt.float32, mybir.dt.float16, mybir.dt.int16
    bf16 = mybir.dt.bfloat16
    Sin = mybir.ActivationFunctionType.Sin
    Copy = mybir.ActivationFunctionType.Copy
    Abs = mybir.ActivationFunctionType.Abs
    mult = mybir.AluOpType.mult
    add = mybir.AluOpType.add
    sub_ = mybir.AluOpType.subtract
    maxop = mybir.AluOpType.max
    ntiles = bs // 128

    nc = bacc.Bacc("TRN2", target_bir_lowering=False, debug=False)
    x_ext = nc.declare_dram_parameter("x16", [bs, PIX], f16, isOutput=False)
    ia_ext = nc.declare_dram_parameter("ia", [bs, PIX], i16, isOutput=False)
    cc_ext = nc.declare_dram_parameter("cc", [bs, rounds * NG], i16, isOutput=False)
    th_ext = nc.declare_dram_parameter("theta", [128, C], f32, isOutput=False)
    out_ext = nc.declare_dram_parameter("out", [bs, OUTW], f16, isOutput=True)

    PI, TWO_PI = math.pi, 2 * math.pi
    MAGIC, INV2PI = 1.5 * 2 ** 23, 1.0 / (2 * math.pi)

    with tile.TileContext(nc) as tc, ExitStack() as ctx:
        cpool = ctx.enter_context(tc.tile_pool(name="const", bufs=1))
        xpool = ctx.enter_context(tc.tile_pool(name="xf", bufs=2))
        ipool = ctx.enter_context(tc.tile_pool(name="ia", bufs=2))
        kpool = ctx.enter_context(tc.tile_pool(name="cc", bufs=2))
        spool = ctx.enter_context(tc.tile_pool(name="sc", bufs=2))
        upool = ctx.enter_context(tc.tile_pool(name="uc", bufs=2))
        vpool = ctx.enter_context(tc.tile_pool(name="v", bufs=2))
        tpool = ctx.enter_context(tc.tile_pool(name="trig", bufs=2))
        wpool = ctx.enter_context(tc.tile_pool(name="we", bufs=2))
        epool = ctx.enter_context(tc.tile_pool(name="expand", bufs=2))
        opool = ctx.enter_context(tc.tile_pool(name="ot", bufs=2))

        pihalf = cpool.tile([128, 1], f32)
        nc.vector.memset(pihalf[:], PI / 2)
        zerob = cpool.tile([128, 1], f32)
        nc.vector.memset(zerob[:], 0.0)

        # Scalar-engine Sin only accepts [-pi, pi]. Range-reduce with the
        # round-to-nearest magic trick: n = (v/2pi + M) - M, -r = 2pi*n - v.
        # Then -sin(v) = Sin(-r) and cos(v) = Sin(pi/2 - |r|); the sin sign
        # flip cancels in sin*sin products and is absorbed into nhst.
        def trig(pool, src, width, tagp, on_act):
            """returns (cv, svN) = (cos(src), -sin(src)), width cols."""
            t1 = pool.tile([128, width], f32, tag=tagp + "t1")
            if on_act:
                nc.scalar.activation(t1[:], src, Copy, bias=MAGIC, scale=INV2PI)
                nc.scalar.activation(t1[:], t1[:], Copy, bias=-MAGIC, scale=1.0)
            else:
                nc.vector.tensor_scalar(t1[:], src, INV2PI, MAGIC, mult, add)
                nc.vector.tensor_scalar(t1[:], t1[:], MAGIC, None, sub_)
            negr = pool.tile([128, width], f16, tag=tagp + "negr")
            nc.vector.scalar_tensor_tensor(negr[:], t1[:], TWO_PI, src, mult, sub_)
            absr = pool.tile([128, width], f16, tag=tagp + "absr")
            nc.vector.tensor_scalar(absr[:], negr[:], -1.0, None, mult)
            nc.vector.tensor_tensor(absr[:], absr[:], negr[:], maxop)  # |r|
            cv = pool.tile([128, width], f16, tag=tagp + "cv")
            svN = pool.tile([128, width], f16, tag=tagp + "svN")
            nc.scalar.activation(svN[:], negr[:], Sin, bias=zerob[:, 0:1])
            nc.scalar.activation(cv[:], absr[:], Sin, bias=pihalf[:, 0:1], scale=-1.0)
            return cv, svN

        # theta coefficients: hct = 0.5*cos(theta), nhst = -0.5*sin(theta)
        th_sb = cpool.tile([128, C], f32)
        nc.sync.dma_start(out=th_sb[:], in_=th_ext[:, :])
        tt1 = cpool.tile([128, C], f32)
        nc.vector.tensor_scalar(tt1[:], th_sb[:], INV2PI, MAGIC, mult, add)
        nc.vector.tensor_scalar(tt1[:], tt1[:], MAGIC, None, sub_)
        tnegr = cpool.tile([128, C], f32)
        nc.vector.scalar_tensor_tensor(tnegr[:], tt1[:], TWO_PI, th_sb[:], mult, sub_)
        nc.vector.tensor_scalar(tt1[:], tnegr[:], -1.0, None, mult)
        nc.vector.tensor_tensor(tt1[:], tt1[:], tnegr[:], maxop)
        cvt = cpool.tile([128, C], f32)
        svNt = cpool.tile([128, C], f32)
        nc.scalar.activation(svNt[:], tnegr[:], Sin, bias=zerob[:, 0:1])
        nc.scalar.activation(cvt[:], tt1[:], Sin, bias=pihalf[:, 0:1], scale=-1.0)
        hcoef = cpool.tile([128, 2 * C], f32)
        nc.vector.tensor_scalar(hcoef[:, 0:C], cvt[:], 0.5, None, mult)
        nc.vector.tensor_scalar(hcoef[:, C:2 * C], svNt[:], 0.5, None, mult)
        hct = hcoef[:, 0:C]        # 0.5*cos(theta)
        nhst = hcoef[:, C:2 * C]   # -0.5*sin(theta) = 0.5*svN

        # pair-major replicated theta tables (one-time, via ACT): unit
        # stride operands let the per-tile products hit the 16-bit 2x mode
        hrep = cpool.tile([128, NPAIR * C], bf16)
        nrep = cpool.tile([128, NPAIR * C], bf16)
        nc.scalar.activation(
            hrep[:].rearrange("p (a b) -> p a b", b=C),
            hct.unsqueeze(1).broadcast_to([128, NPAIR, C]), Copy,
        )
        nc.scalar.activation(
            nrep[:].rearrange("p (a b) -> p a b", b=C),
            nhst.unsqueeze(1).broadcast_to([128, NPAIR, C]), Copy,
        )

        for t in range(ntiles):
            rows = slice(t * 128, (t + 1) * 128)
            ia = ipool.tile([128, PIX], i16)
            xf = xpool.tile([128, PIX], f16)
            if t == 0:
                # split the first tile's DMAs and round-0 scatter into
                # pixel halves so work starts before the full tile lands
                HX = PIX // 2
                nc.sync.dma_start(out=ia[:, 0:HX], in_=ia_ext[rows, 0:HX])
                nc.sync.dma_start(out=xf[:, 0:HX], in_=x_ext[rows, 0:HX])
                nc.sync.dma_start(out=ia[:, HX:PIX], in_=ia_ext[rows, HX:PIX])
                nc.sync.dma_start(out=xf[:, HX:PIX], in_=x_ext[rows, HX:PIX])
            else:
                nc.sync.dma_start(out=ia[:], in_=ia_ext[rows, :])
                nc.sync.dma_start(out=xf[:], in_=x_ext[rows, :])
            ct = kpool.tile([128, rounds * NG], i16)
            nc.sync.dma_start(out=ct[:], in_=cc_ext[rows, :])

            # log-doubling scatter rounds; U accumulates (disjoint supports)
            S = spool.tile([128, (rounds + 1) * NG], f16)
            U = upool.tile([128, rounds * NG], f16)
            if t == 0:
                HX = PIX // 2
                s0b = vpool.tile([128, NG], f16, tag="s0b")
                nc.gpsimd.local_scatter(
                    S[:, 0:NG], xf[:, 0:HX], ia[:, 0:HX],
                    channels=128, num_elems=NG, num_idxs=HX,
                )
                nc.gpsimd.local_scatter(
                    s0b[:], xf[:, HX:PIX], ia[:, HX:PIX],
                    channels=128, num_elems=NG, num_idxs=HX,
                )
                nc.vector.tensor_tensor(S[:, 0:NG], S[:, 0:NG], s0b[:], add)
            else:
                nc.gpsimd.local_scatter(
                    S[:, 0:NG], xf[:], ia[:],
                    channels=128, num_elems=NG, num_idxs=PIX,
                )
            V = vpool.tile([128, NG], f16)
            for r in range(1, rounds + 1):
                src = S[:, 0:NG] if r == 1 else U[:, (r - 2) * NG:(r - 1) * NG]
                nc.gpsimd.local_scatter(
                    S[:, r * NG:(r + 1) * NG], src,
                    ct[:, (r - 1) * NG:(r - 1) * NG + NG],
                    channels=128, num_elems=NG, num_idxs=NG,
                )
                dst = V[:] if r == rounds else U[:, (r - 1) * NG:r * NG]
                nc.vector.tensor_tensor(dst, src, S[:, r * NG:(r + 1) * NG], add)

            cv, sv = trig(tpool, V[:], NG, "g", True)

            # half-split layout: slots [0:460] = x0, [460:920] = x1
            w = wpool.tile([128, NPAIR], f16, tag="w")
            e = wpool.tile([128, NPAIR], f16, tag="e")
            nc.vector.tensor_tensor(w[:], sv[:, 0:NPAIR], sv[:, NPAIR:NG], mult)
            nc.vector.tensor_tensor(e[:], cv[:, 0:NPAIR], cv[:, NPAIR:NG], mult)

            # class expansion: even = (A*hct_c + 0.5) + W*nhst_c, odd = 0.5*E+0.5
            # ACT (non-contending) replicates A and W pair-major; the DVE
            # products are then all-unit-stride 16-bit (2x mode), in-place.
            # Processed in pair-halves so ACT/DVE/DMA pipeline in the drain.
            tev = epool.tile([128, NPAIR * C], bf16, tag="tev")
            tw2 = epool.tile([128, NPAIR * C], bf16, tag="tw2")
            ot = opool.tile([128, OUTW], f16)
            otv = ot[:].rearrange("p (a b) -> p a b", b=2 * C)
            HP = NPAIR // 2
            for h in range(2):
                ph = slice(h * HP, (h + 1) * HP)
                pc = slice(h * HP * C, (h + 1) * HP * C)
                tev3 = tev[:, pc].rearrange("p (a b) -> p a b", b=C)
                tw23 = tw2[:, pc].rearrange("p (a b) -> p a b", b=C)
                A3 = cv[:, ph].unsqueeze(2).broadcast_to([128, HP, C])
                W3 = w[:, ph].unsqueeze(2).broadcast_to([128, HP, C])
                E3 = e[:, ph].unsqueeze(2).broadcast_to([128, HP, C])
                nc.scalar.activation(tev3, A3, Copy)
                nc.vector.tensor_tensor(tev[:, pc], tev[:, pc], hrep[:, pc], mult)
                nc.scalar.activation(tw23, W3, Copy)
                nc.vector.tensor_tensor(tw2[:, pc], tw2[:, pc], nrep[:, pc], mult)
                nc.vector.scalar_tensor_tensor(
                    otv[:, ph, 0:C], tev3, 0.5, tw23, add, add
                )
                nc.scalar.activation(otv[:, ph, C:2 * C], E3, Copy, bias=0.5, scale=0.5)
                nc.sync.dma_start(
                    out=out_ext[rows, h * HP * 2 * C:(h + 1) * HP * 2 * C],
                    in_=ot[:, h * HP * 2 * C:(h + 1) * HP * 2 * C],
                )

    nc.compile()
    return nc


def _prep_scatter_maps(pair_idx):
    """Build round-0 scatter map and log-doubling duplicate maps.

    Slot layout is half-split: x0 of pair p -> slot p, x1 -> slot 460+p.
    Chain round 0 serves ordinal 1, round 1 serves ordinals 2-3 (via the
    running union), round 2 serves 4-7, etc.
    Returns (idxA [B, PIX] i16, chains [T, B, NG] i16, T).
    """
    pidx = pair_idx.reshape(B, NPAIR, 2)
    idx = np.concatenate([pidx[:, :, 0], pidx[:, :, 1]], axis=1).astype(np.int64)
    j = np.arange(NG, dtype=np.int64)[None, :]
    ordk = np.argsort(idx * 1024 + j, axis=1)      # slots sorted by (pixel, slot)
    px_sorted = np.take_along_axis(idx, ordk, axis=1)
    first = np.ones((B, NG), dtype=bool)
    first[:, 1:] = px_sorted[:, 1:] != px_sorted[:, :-1]
    kk = np.broadcast_to(np.arange(NG, dtype=np.int64), (B, NG))
    run_start = np.maximum.accumulate(np.where(first, kk, 0), axis=1)
    o = kk - run_start                              # occurrence ordinal per sorted pos
    maxmult = int(o.max()) + 1
    T = max(1, (maxmult - 1).bit_length())          # ceil(log2(maxmult))

    idxA = np.full((B, PIX), -1, np.int16)
    rr, cc = np.nonzero(first)
    idxA[rr, px_sorted[rr, cc]] = ordk[rr, cc]

    chains = np.full((T, B, NG), -1, np.int16)
    rr, cc = np.nonzero(o >= 1)
    oo = o[rr, cc]
    t_of = np.zeros_like(oo)
    for t in range(1, T):
        t_of[oo >= (1 << t)] = t
    src = ordk[rr, cc - (1 << t_of)]
    dst = ordk[rr, cc]
    chains[t_of, rr, src] = dst
    return idxA, chains, T


def _get_nc(rounds):
    key = ("nc", rounds)
    if key not in _cache:
        _cache[key] = build_nc(rounds=rounds)
    return _cache[key]


def kernel(x, pair_idx, theta):
    _ensure_path()
    from concourse.bass_utils import run_bass_kernel_spmd

    x16 = np.ascontiguousarray(
        np.asarray(x, dtype=np.float32).reshape(B, PIX).astype(np.float16)
    )
    idxA, chains, T = _prep_scatter_maps(np.asarray(pair_idx))
    nc = _get_nc(T)
    cc = np.ascontiguousarray(
        chains.transpose(1, 0, 2).reshape(B, T * NG)
    )
    thb = np.ascontiguousarray(
        np.tile(np.asarray(theta, dtype=np.float32).reshape(1, C), (128, 1))
    )
    in_maps = [
        {
            "x16": x16[k * BS:(k + 1) * BS],
            "ia": idxA[k * BS:(k + 1) * BS],
            "cc": cc[k * BS:(k + 1) * BS],
            "theta": thb,
        }
        for k in range(NCORES)
    ]
    res = run_bass_kernel_spmd(nc, in_maps, list(range(NCORES))).results
    out = np.concatenate(
        [res[k]["out"].astype(np.float32) for k in range(NCORES)], axis=0
    )
    return out.reshape(B, NG, C)
